# revision 12
# baseline (speedup 1.0000x reference)
"""ComplexUnPooling2D scatter kernel for 8 Trainium2 NeuronCores.

Reference semantics: out_flat = zeros(4*n); out_flat[unpool_mat.ravel()] = inputs.ravel()
where unpool_mat[i] = 4*i + off_i, off_i in [0,4)  (2x2 maxpool argmax structure,
indices strictly increasing, batch-local).  Hence, viewing the output as [n, 4]:

    out[i, j] = inputs[i] * ((unpool_mat[i] & 3) == j)

which is a pure streaming elementwise op — no indirect scatter needed.

Sharding: batch dim across 8 cores (2 batches/core).  The kernel only ever
needs the LOW 32-bit word of each (little-endian) int64 index, so the host
passes, per core, a single fused int32 tensor per tile row block:
columns [0:F) = the f32 input bits, columns [F:2F) = the low index words.
Device does all arithmetic: AND, one-hot compares, multiplies, interleave.

Engine split per tile: DVE does the AND + fused (off==j)*x for j=0,1 plus the
j=2,3 masks; gpsimd (Pool) does the j=2,3 multiplies.  Input DMAs ride the
Activation-engine HWDGE ring, output DMAs the sync ring (separate FIFO sets,
no head-of-line blocking).
"""
import sys

sys.path.insert(0, "/opt/trn_rl_repo")

import numpy as np

import concourse.bacc as bacc
import concourse.mybir as mybir
import concourse.tile as tile
from concourse.bass_utils import run_bass_kernel_spmd

# Problem constants (hardcoded per contract)
B, H, W, C = 16, 64, 64, 128
OUT_SHAPE = (B, 2 * H, 2 * W, C)
N_CORES = 8
N_PER_CORE = (B // N_CORES) * H * W * C  # 1,048,576 elements
P = 128  # SBUF partitions

# Tiling: input viewed per-core as [T*P, F]
F = 1024
T = N_PER_CORE // (P * F)  # 8
assert T * P * F == N_PER_CORE


def _build_program():
    # Bacc (not raw Bass): its compile() runs generate_event_semaphores,
    # which splits multi-sem waits (TRN2 allows max 1 wait per instruction).
    nc = bacc.Bacc(
        "TRN2",
        target_bir_lowering=False,
        debug=False,
        num_devices=N_CORES,
    )
    # fused per-tile rows: [x bits (F words) | low index words (F words)]
    fx = nc.dram_tensor("fx", [T * P, 2 * F], mybir.dt.int32, kind="ExternalInput").ap()
    y = nc.dram_tensor("y", [T * P, 4 * F], mybir.dt.float32, kind="ExternalOutput").ap()

    AL = mybir.AluOpType
    with tile.TileContext(nc) as tc:
        with (
            tc.tile_pool(name="pin", bufs=6) as pin,
            tc.tile_pool(name="pout", bufs=4) as pout,
        ):
            for t in range(T):
                rows = slice(t * P, (t + 1) * P)
                ft = pin.tile([P, 2 * F], mybir.dt.int32, tag="f")
                off = pin.tile([P, F], mybir.dt.int32, tag="off")
                m2 = pin.tile([P, F], mybir.dt.float32, tag="m2")
                m3 = pin.tile([P, F], mybir.dt.float32, tag="m3")
                ot = pout.tile([P, 4 * F], mybir.dt.float32, tag="out")
                nc.scalar.dma_start(out=ft[:], in_=fx[rows, :])
                xv = ft[:, 0:F].bitcast(mybir.dt.float32)
                lo = ft[:, F : 2 * F]
                # off = lo & 3  (int-domain; bitwise ops can't cast on write)
                nc.vector.tensor_scalar(
                    out=off[:], in0=lo, scalar1=3, scalar2=None,
                    op0=AL.bitwise_and,
                )
                o4 = ot.rearrange("p (f four) -> p f four", four=4)
                # j=0,1: fused (off==j)*x on DVE (strided write)
                for j in range(2):
                    nc.vector.scalar_tensor_tensor(
                        out=o4[:, :, j], in0=off[:], scalar=float(j), in1=xv,
                        op0=AL.is_equal, op1=AL.mult,
                    )
                # j=2,3: DVE builds masks contiguously; gpsimd multiplies into
                # the strided slots — splits the element work across engines.
                for j, m in ((2, m2), (3, m3)):
                    nc.vector.tensor_scalar(
                        out=m[:], in0=off[:], scalar1=j, scalar2=None,
                        op0=AL.is_equal,
                    )
                    nc.gpsimd.tensor_tensor(
                        out=o4[:, :, j], in0=m[:], in1=xv, op=AL.mult,
                    )
                nc.sync.dma_start(out=y[rows, :], in_=ot[:])
    nc.compile()
    return nc


_NC_CACHE = None


def _get_program():
    global _NC_CACHE
    if _NC_CACHE is None:
        _NC_CACHE = _build_program()
    return _NC_CACHE


def _low_words(idx: np.ndarray) -> np.ndarray:
    """Low 32-bit word of each index (indices fit in int32: max 4n < 2^31)."""
    flat = np.ascontiguousarray(idx).reshape(-1)
    if flat.dtype == np.int64:
        return np.ascontiguousarray(flat.view(np.int32).reshape(-1, 2)[:, 0])
    return flat.astype(np.int32, copy=False)


def _make_in_maps(inputs: np.ndarray, unpool_mat: np.ndarray):
    bpc = B // N_CORES  # batches per core
    in_maps = []
    for c in range(N_CORES):
        sl = slice(c * bpc, (c + 1) * bpc)
        xb = np.ascontiguousarray(inputs[sl]).reshape(T * P, F).view(np.int32)
        lw = _low_words(unpool_mat[sl]).reshape(T * P, F)
        fused = np.empty((T * P, 2 * F), dtype=np.int32)
        fused[:, :F] = xb
        fused[:, F:] = lw
        in_maps.append({"fx": fused})
    return in_maps


def kernel(inputs, unpool_mat, output_shape=None, **_unused):
    inputs = np.asarray(inputs)
    unpool_mat = np.asarray(unpool_mat)
    assert inputs.shape == (B, H, W, C), inputs.shape
    if output_shape is not None:
        assert tuple(int(s) for s in np.asarray(output_shape).reshape(-1)) == OUT_SHAPE

    nc = _get_program()
    in_maps = _make_in_maps(inputs, unpool_mat)
    res = run_bass_kernel_spmd(nc, in_maps, core_ids=list(range(N_CORES)))
    bpc = B // N_CORES
    out = np.concatenate(
        [r["y"].reshape(bpc, 2 * H, 2 * W, C) for r in res.results], axis=0
    )
    return out


# revision 13
# speedup vs baseline: 1.1487x; 1.1487x over previous
"""ComplexUnPooling2D scatter kernel for 8 Trainium2 NeuronCores.

Reference semantics: out_flat = zeros(4*n); out_flat[unpool_mat.ravel()] = inputs.ravel()
where unpool_mat[i] = 4*i + off_i, off_i in [0,4)  (2x2 maxpool argmax structure,
indices strictly increasing, batch-local).  Hence, viewing the output as [n, 4]:

    out[i, j] = inputs[i] * ((unpool_mat[i] & 3) == j)

which is a pure streaming elementwise op — no indirect scatter needed.

Sharding: batch dim across 8 cores (2 batches/core).  The kernel only ever
needs the LOW 32-bit word of each (little-endian) int64 index, so the host
passes, per core, a single fused int32 tensor per tile row block:
columns [0:F) = the f32 input bits, columns [F:2F) = the low index words.
Device does all arithmetic: AND, one-hot compares, multiplies, interleave.

Engine split per tile: DVE does the AND + fused (off==j)*x for j=0,1 plus the
j=2,3 masks; gpsimd (Pool) does the j=2,3 multiplies.  Input DMAs ride the
Activation-engine HWDGE ring, output DMAs the sync ring (separate FIFO sets,
no head-of-line blocking).
"""
import sys

sys.path.insert(0, "/opt/trn_rl_repo")

import numpy as np

import concourse.bacc as bacc
import concourse.mybir as mybir
import concourse.tile as tile
from concourse.bass_utils import run_bass_kernel_spmd

# Problem constants (hardcoded per contract)
B, H, W, C = 16, 64, 64, 128
OUT_SHAPE = (B, 2 * H, 2 * W, C)
N_CORES = 8
N_PER_CORE = (B // N_CORES) * H * W * C  # 1,048,576 elements
P = 128  # SBUF partitions

# Tiling: input viewed per-core as [T*P, F]
F = 1024
T = N_PER_CORE // (P * F)  # 8
assert T * P * F == N_PER_CORE


def _build_program():
    # Bacc (not raw Bass): its compile() runs generate_event_semaphores,
    # which splits multi-sem waits (TRN2 allows max 1 wait per instruction).
    nc = bacc.Bacc(
        "TRN2",
        target_bir_lowering=False,
        debug=False,
        num_devices=N_CORES,
    )
    # fused per-tile rows: [x bits (F words) | low index words (F words)]
    fx = nc.dram_tensor("fx", [T * P, 2 * F], mybir.dt.int32, kind="ExternalInput").ap()
    y = nc.dram_tensor("y", [T * P, 4 * F], mybir.dt.float32, kind="ExternalOutput").ap()

    AL = mybir.AluOpType
    with tile.TileContext(nc) as tc:
        with (
            tc.tile_pool(name="pin", bufs=6) as pin,
            tc.tile_pool(name="pout", bufs=4) as pout,
        ):
            for t in range(T):
                rows = slice(t * P, (t + 1) * P)
                ft = pin.tile([P, 2 * F], mybir.dt.int32, tag="f")
                off = pin.tile([P, F], mybir.dt.int32, tag="off")
                ot = pout.tile([P, 4 * F], mybir.dt.float32, tag="out")
                nc.scalar.dma_start(out=ft[:], in_=fx[rows, :])
                xv = ft[:, 0:F].bitcast(mybir.dt.float32)
                lo = ft[:, F : 2 * F]
                # off = lo & 3  (int-domain; bitwise ops can't cast on write)
                nc.vector.tensor_scalar(
                    out=off[:], in0=lo, scalar1=3, scalar2=None,
                    op0=AL.bitwise_and,
                )
                o4 = ot.rearrange("p (f four) -> p f four", four=4)
                # all-DVE: gpsimd shares SBUF ports with DVE, so offloading
                # there slows DVE ~2x on shared tiles (measured) — net loss.
                for j in range(4):
                    nc.vector.scalar_tensor_tensor(
                        out=o4[:, :, j], in0=off[:], scalar=float(j), in1=xv,
                        op0=AL.is_equal, op1=AL.mult,
                    )
                nc.sync.dma_start(out=y[rows, :], in_=ot[:])
    nc.compile()
    return nc


_NC_CACHE = None


def _get_program():
    global _NC_CACHE
    if _NC_CACHE is None:
        _NC_CACHE = _build_program()
    return _NC_CACHE


def _low_words(idx: np.ndarray) -> np.ndarray:
    """Low 32-bit word of each index (indices fit in int32: max 4n < 2^31)."""
    flat = np.ascontiguousarray(idx).reshape(-1)
    if flat.dtype == np.int64:
        return np.ascontiguousarray(flat.view(np.int32).reshape(-1, 2)[:, 0])
    return flat.astype(np.int32, copy=False)


def _make_in_maps(inputs: np.ndarray, unpool_mat: np.ndarray):
    bpc = B // N_CORES  # batches per core
    in_maps = []
    for c in range(N_CORES):
        sl = slice(c * bpc, (c + 1) * bpc)
        xb = np.ascontiguousarray(inputs[sl]).reshape(T * P, F).view(np.int32)
        lw = _low_words(unpool_mat[sl]).reshape(T * P, F)
        fused = np.empty((T * P, 2 * F), dtype=np.int32)
        fused[:, :F] = xb
        fused[:, F:] = lw
        in_maps.append({"fx": fused})
    return in_maps


def kernel(inputs, unpool_mat, output_shape=None, **_unused):
    inputs = np.asarray(inputs)
    unpool_mat = np.asarray(unpool_mat)
    assert inputs.shape == (B, H, W, C), inputs.shape
    if output_shape is not None:
        assert tuple(int(s) for s in np.asarray(output_shape).reshape(-1)) == OUT_SHAPE

    nc = _get_program()
    in_maps = _make_in_maps(inputs, unpool_mat)
    res = run_bass_kernel_spmd(nc, in_maps, core_ids=list(range(N_CORES)))
    bpc = B // N_CORES
    out = np.concatenate(
        [r["y"].reshape(bpc, 2 * H, 2 * W, C) for r in res.results], axis=0
    )
    return out


# revision 16
# speedup vs baseline: 1.1695x; 1.0181x over previous
"""ComplexUnPooling2D scatter kernel for 8 Trainium2 NeuronCores.

Reference semantics: out_flat = zeros(4*n); out_flat[unpool_mat.ravel()] = inputs.ravel()
where unpool_mat[i] = 4*i + off_i, off_i in [0,4)  (2x2 maxpool argmax structure,
indices strictly increasing, batch-local).  Hence, viewing the output as [n, 4]:

    out[i, j] = inputs[i] * ((unpool_mat[i] & 3) == j)

which is a pure streaming elementwise op — no indirect scatter needed.

Sharding: batch dim across 8 cores (2 batches/core).  The kernel only ever
needs the LOW 32-bit word of each (little-endian) int64 index, so the host
passes, per core, a single fused int32 tensor per tile row block:
columns [0:F) = the f32 input bits, columns [F:2F) = the low index words.
Device does all arithmetic: AND, one-hot compares, multiplies, interleave.

Engine split per tile: DVE does the AND + fused (off==j)*x for j=0,1 plus the
j=2,3 masks; gpsimd (Pool) does the j=2,3 multiplies.  Input DMAs ride the
Activation-engine HWDGE ring, output DMAs the sync ring (separate FIFO sets,
no head-of-line blocking).
"""
import sys

sys.path.insert(0, "/opt/trn_rl_repo")

import numpy as np

import concourse.bacc as bacc
import concourse.dve_ops as dve_ops
import concourse.mybir as mybir
import concourse.tile as tile
from concourse.bass_utils import run_bass_kernel_spmd
from concourse.dve_spec import Spec, Src0, Src1, Zero, Idx, eq, select
from concourse.dve_spec import lower as dve_lower
from concourse.dve_uop import DveOpSpec

# Problem constants (hardcoded per contract)
B, H, W, C = 16, 64, 64, 128
OUT_SHAPE = (B, 2 * H, 2 * W, C)
N_CORES = 8
N_PER_CORE = (B // N_CORES) * H * W * C  # 1,048,576 elements
P = 128  # SBUF partitions

# Tiling: input viewed per-core as [T*P, F]
F = 1024
T = N_PER_CORE // (P * F)  # 8
assert T * P * F == N_PER_CORE

# --- custom DVE op: the whole one-hot expand-multiply in one instruction ---
# out[p, c] = x[p, c>>2] * (q[p, c>>2] == c), where q = lo & (4F-1) = 4f+off
# is each input element's target position within its row's 4F output run.
# Inputs stream via broadcast APs (each element repeated 4x); Idx is the
# implicit output element counter.  One pass over the output domain replaces
# four strided scalar_tensor_tensor ops (~8.6us -> ~4.7us per tile on DVE).
_OP_NAME = "UNPOOL_ONEHOT_MUL_ANT"


def _register_unpool_op():
    for o in dve_ops.OPS:
        if o.name == _OP_NAME:
            return o

    def _ref(in0, in1, s0, s1, imm2):
        p = in0.shape[0]
        a = in0.reshape(p, -1).astype(np.float32)
        b = in1.reshape(p, -1).astype(np.float32)
        idx = np.arange(a.shape[1], dtype=np.float32)[None, :]
        return np.where(a == idx, b, np.float32(0.0)).astype(np.float32)

    spec = Spec(body=select(eq(Src0, Idx), Src1, Zero), reference=_ref)
    row = max(dve_ops._SUB_OPCODE_FOR_NAME.values()) + 1
    assert row < 0x20, row
    dve_ops._SUB_OPCODE_FOR_NAME[_OP_NAME] = row
    shas = {}
    for ver in ("v3", "v4"):
        s = DveOpSpec(
            name=_OP_NAME, opcode=row, uops=dve_lower(spec, ver=ver), rd1_en=True
        )
        shas[ver] = s.sha(ver)
    op = dve_ops.DveOp(_OP_NAME, spec, subdim=False, uops_sha=shas)
    dve_ops.OPS.append(op)
    dve_ops.CUSTOM_DVE_SPECS[_OP_NAME] = op.spec
    return op


_UNPOOL_OP = _register_unpool_op()


def _build_program():
    # Bacc (not raw Bass): its compile() runs generate_event_semaphores,
    # which splits multi-sem waits (TRN2 allows max 1 wait per instruction).
    nc = bacc.Bacc(
        "TRN2",
        target_bir_lowering=False,
        debug=False,
        num_devices=N_CORES,
    )
    # fused per-tile rows: [x bits (F words) | low index words (F words)]
    fx = nc.dram_tensor("fx", [T * P, 2 * F], mybir.dt.int32, kind="ExternalInput").ap()
    y = nc.dram_tensor("y", [T * P, 4 * F], mybir.dt.float32, kind="ExternalOutput").ap()

    AL = mybir.AluOpType
    with tile.TileContext(nc) as tc:
        with (
            tc.tile_pool(name="pin", bufs=6) as pin,
            tc.tile_pool(name="pout", bufs=4) as pout,
        ):
            for t in range(T):
                rows = slice(t * P, (t + 1) * P)
                ft = pin.tile([P, 2 * F], mybir.dt.int32, tag="f")
                off = pin.tile([P, F], mybir.dt.int32, tag="off")
                ot = pout.tile([P, 4 * F], mybir.dt.float32, tag="out")
                nc.scalar.dma_start(out=ft[:], in_=fx[rows, :])
                xv = ft[:, 0:F].bitcast(mybir.dt.float32)
                lo = ft[:, F : 2 * F]
                # q = lo & (4F-1) = 4f + off  (int-domain AND)
                nc.vector.tensor_scalar(
                    out=off[:], in0=lo, scalar1=4 * F - 1, scalar2=None,
                    op0=AL.bitwise_and,
                )
                q_b = off[:].unsqueeze(2).to_broadcast([P, F, 4])
                x_b = xv.unsqueeze(2).to_broadcast([P, F, 4])
                nc.vector._custom_dve(_UNPOOL_OP, out=ot[:], in0=q_b, in1=x_b)
                nc.sync.dma_start(out=y[rows, :], in_=ot[:])
    nc.compile()
    return nc


_NC_CACHE = None


def _get_program():
    global _NC_CACHE
    if _NC_CACHE is None:
        _NC_CACHE = _build_program()
    return _NC_CACHE


def _low_words(idx: np.ndarray) -> np.ndarray:
    """Low 32-bit word of each index (indices fit in int32: max 4n < 2^31)."""
    flat = np.ascontiguousarray(idx).reshape(-1)
    if flat.dtype == np.int64:
        return np.ascontiguousarray(flat.view(np.int32).reshape(-1, 2)[:, 0])
    return flat.astype(np.int32, copy=False)


def _make_in_maps(inputs: np.ndarray, unpool_mat: np.ndarray):
    bpc = B // N_CORES  # batches per core
    in_maps = []
    for c in range(N_CORES):
        sl = slice(c * bpc, (c + 1) * bpc)
        xb = np.ascontiguousarray(inputs[sl]).reshape(T * P, F).view(np.int32)
        lw = _low_words(unpool_mat[sl]).reshape(T * P, F)
        fused = np.empty((T * P, 2 * F), dtype=np.int32)
        fused[:, :F] = xb
        fused[:, F:] = lw
        in_maps.append({"fx": fused})
    return in_maps


def kernel(inputs, unpool_mat, output_shape=None, **_unused):
    inputs = np.asarray(inputs)
    unpool_mat = np.asarray(unpool_mat)
    assert inputs.shape == (B, H, W, C), inputs.shape
    if output_shape is not None:
        assert tuple(int(s) for s in np.asarray(output_shape).reshape(-1)) == OUT_SHAPE

    nc = _get_program()
    in_maps = _make_in_maps(inputs, unpool_mat)
    res = run_bass_kernel_spmd(nc, in_maps, core_ids=list(range(N_CORES)))
    bpc = B // N_CORES
    out = np.concatenate(
        [r["y"].reshape(bpc, 2 * H, 2 * W, C) for r in res.results], axis=0
    )
    return out


# revision 18
# speedup vs baseline: 1.4793x; 1.2649x over previous
"""ComplexUnPooling2D scatter kernel for 8 Trainium2 NeuronCores.

Reference semantics: out_flat = zeros(4*n); out_flat[unpool_mat.ravel()] = inputs.ravel()
where unpool_mat[i] = 4*i + off_i, off_i in [0,4)  (2x2 maxpool argmax structure,
indices strictly increasing, batch-local).  Hence, viewing the output as [n, 4]:

    out[i, j] = inputs[i] * ((unpool_mat[i] & 3) == j)

which is a pure streaming elementwise op — no indirect scatter needed.

Sharding: batch dim across 8 cores (2 batches/core).  The kernel only ever
needs the LOW 32-bit word of each (little-endian) int64 index, so the host
passes, per core, a single fused int32 tensor per tile row block:
columns [0:F) = the f32 input bits, columns [F:2F) = the low index words.
Device does all arithmetic: AND, one-hot compares, multiplies, interleave.

Engine split per tile: DVE does the AND + fused (off==j)*x for j=0,1 plus the
j=2,3 masks; gpsimd (Pool) does the j=2,3 multiplies.  Input DMAs ride the
Activation-engine HWDGE ring, output DMAs the sync ring (separate FIFO sets,
no head-of-line blocking).
"""
import sys

sys.path.insert(0, "/opt/trn_rl_repo")

import numpy as np

import concourse.bacc as bacc
import concourse.dve_ops as dve_ops
import concourse.mybir as mybir
import concourse.tile as tile
from concourse.bass_utils import run_bass_kernel_spmd
from concourse.dve_spec import Spec, Src0, Src1, Zero, Idx, eq, select
from concourse.dve_spec import lower as dve_lower
from concourse.dve_uop import DveOpSpec

# Problem constants (hardcoded per contract)
B, H, W, C = 16, 64, 64, 128
OUT_SHAPE = (B, 2 * H, 2 * W, C)
N_CORES = 8
N_PER_CORE = (B // N_CORES) * H * W * C  # 1,048,576 elements
P = 128  # SBUF partitions

# Tiling: input viewed per-core as [T*P, F]
F = 1024
T = N_PER_CORE // (P * F)  # 8
assert T * P * F == N_PER_CORE

# --- custom DVE op: the whole one-hot expand-multiply in one instruction ---
# out[p, c] = x[p, c>>2] * (q[p, c>>2] == c), where q = lo & (4F-1) = 4f+off
# is each input element's target position within its row's 4F output run.
# Inputs stream via broadcast APs (each element repeated 4x); Idx is the
# implicit output element counter.  One pass over the output domain replaces
# four strided scalar_tensor_tensor ops (~8.6us -> ~4.7us per tile on DVE).
_OP_NAME = "UNPOOL_ONEHOT_MUL_ANT"


def _register_unpool_op():
    for o in dve_ops.OPS:
        if o.name == _OP_NAME:
            return o

    def _ref(in0, in1, s0, s1, imm2):
        p = in0.shape[0]
        a = in0.reshape(p, -1).astype(np.float32)
        b = in1.reshape(p, -1).astype(np.float32)
        idx = np.arange(a.shape[1], dtype=np.float32)[None, :]
        return np.where(a == idx, b, np.float32(0.0)).astype(np.float32)

    spec = Spec(body=select(eq(Src0, Idx), Src1, Zero), reference=_ref)
    row = max(dve_ops._SUB_OPCODE_FOR_NAME.values()) + 1
    assert row < 0x20, row
    dve_ops._SUB_OPCODE_FOR_NAME[_OP_NAME] = row
    shas = {}
    for ver in ("v3", "v4"):
        s = DveOpSpec(
            name=_OP_NAME, opcode=row, uops=dve_lower(spec, ver=ver), rd1_en=True
        )
        shas[ver] = s.sha(ver)
    op = dve_ops.DveOp(_OP_NAME, spec, subdim=False, uops_sha=shas)
    dve_ops.OPS.append(op)
    dve_ops.CUSTOM_DVE_SPECS[_OP_NAME] = op.spec
    return op


_UNPOOL_OP = _register_unpool_op()


def _build_program():
    # Bacc (not raw Bass): its compile() runs generate_event_semaphores,
    # which splits multi-sem waits (TRN2 allows max 1 wait per instruction).
    nc = bacc.Bacc(
        "TRN2",
        target_bir_lowering=False,
        debug=False,
        num_devices=N_CORES,
    )
    # x: the f32 inputs; lo: raw low 16 bits of each int64 index (the kernel
    # needs only idx & (4F-1), and 4F-1 = 4095 fits in the low halfword).
    x = nc.dram_tensor("x", [T * P, F], mybir.dt.float32, kind="ExternalInput").ap()
    lo16 = nc.dram_tensor("lo", [T * P, F], mybir.dt.int16, kind="ExternalInput").ap()
    y = nc.dram_tensor("y", [T * P, 4 * F], mybir.dt.float32, kind="ExternalOutput").ap()

    AL = mybir.AluOpType
    with tile.TileContext(nc) as tc:
        with (
            tc.tile_pool(name="pin", bufs=6) as pin,
            tc.tile_pool(name="pout", bufs=4) as pout,
        ):
            for t in range(T):
                rows = slice(t * P, (t + 1) * P)
                xt = pin.tile([P, F], mybir.dt.float32, tag="x")
                lt = pin.tile([P, F], mybir.dt.int16, tag="lo")
                qt = pin.tile([P, F], mybir.dt.int16, tag="q")
                ot = pout.tile([P, 4 * F], mybir.dt.float32, tag="out")
                nc.scalar.dma_start(out=xt[:], in_=x[rows, :])
                nc.scalar.dma_start(out=lt[:], in_=lo16[rows, :])
                # q = lo & (4F-1) = 4f + off  (int-domain AND, int16)
                nc.vector.tensor_scalar(
                    out=qt[:], in0=lt[:], scalar1=4 * F - 1, scalar2=None,
                    op0=AL.bitwise_and,
                )
                q_b = qt[:].unsqueeze(2).to_broadcast([P, F, 4])
                x_b = xt[:].unsqueeze(2).to_broadcast([P, F, 4])
                nc.vector._custom_dve(_UNPOOL_OP, out=ot[:], in0=q_b, in1=x_b)
                nc.sync.dma_start(out=y[rows, :], in_=ot[:])
    nc.compile()
    return nc


_NC_CACHE = None


def _get_program():
    global _NC_CACHE
    if _NC_CACHE is None:
        _NC_CACHE = _build_program()
    return _NC_CACHE


def _low_halfwords(idx: np.ndarray) -> np.ndarray:
    """Raw low 16 bits of each (little-endian) index word — a byte-level view."""
    flat = np.ascontiguousarray(idx).reshape(-1)
    step = flat.dtype.itemsize // 2  # int64 -> every 4th halfword, int32 -> 2nd
    return np.ascontiguousarray(flat.view(np.int16).reshape(-1, step)[:, 0])


def _make_in_maps(inputs: np.ndarray, unpool_mat: np.ndarray):
    bpc = B // N_CORES  # batches per core
    in_maps = []
    for c in range(N_CORES):
        sl = slice(c * bpc, (c + 1) * bpc)
        in_maps.append(
            {
                "x": np.ascontiguousarray(inputs[sl]).reshape(T * P, F),
                "lo": _low_halfwords(unpool_mat[sl]).reshape(T * P, F),
            }
        )
    return in_maps


def kernel(inputs, unpool_mat, output_shape=None, **_unused):
    inputs = np.asarray(inputs)
    unpool_mat = np.asarray(unpool_mat)
    assert inputs.shape == (B, H, W, C), inputs.shape
    if output_shape is not None:
        assert tuple(int(s) for s in np.asarray(output_shape).reshape(-1)) == OUT_SHAPE

    nc = _get_program()
    in_maps = _make_in_maps(inputs, unpool_mat)
    res = run_bass_kernel_spmd(nc, in_maps, core_ids=list(range(N_CORES)))
    bpc = B // N_CORES
    out = np.concatenate(
        [r["y"].reshape(bpc, 2 * H, 2 * W, C) for r in res.results], axis=0
    )
    return out
